# revision 1
# baseline (speedup 1.0000x reference)
"""Fused attention kernel for Trainium2, SPMD over 8 NeuronCores.

Problem: nn_AttentionFusion (B=8, S1=S2=2048, D1=D2=512, F=256, fp32).

    Q = feat1 @ Wq + bq            [B,S1,F]
    K = feat2 @ Wk + bk            [B,S2,F]
    V = feat2 @ Wv + bv            [B,S2,F]
    A = softmax(Q K^T / sqrt(F))   [B,S1,S2]
    out = (A @ V) @ Wfc + bfc      [B,S1,F]

Sharding: pure data-parallel over batch — core i computes batch element i.

Per-core algorithm (all layouts chosen so no P-matrix transpose is needed):
  1. feat1/feat2 are PE-transposed into [d, s] layout (contraction dim on
     partitions) for the projections.
  2. Q^T, K^T are produced in [f, s] layout; V in natural [s2, f] layout with
     an extra ones-column (col 256).
  3. scores^T [s2, s1] = (K^T)^T-slices @ Q^T directly; exp() is fused into
     the PSUM->SBUF drain (no max subtraction: scores ~ N(0,1), fp32-safe),
     yielding P^T in exactly the layout the PV matmul needs as stationary
     operand.
  4. attn_out [s1, 257] = P^T-slices.T @ V_aug; column 256 is the softmax
     denominator. Normalization is deferred: (P@V)/denom == softmax(P)@V.
  5. attn_out is rescaled by 1/denom, PE-transposed (2 tiles), and multiplied
     by Wfc; bias bfc is added on the way out.
"""

import os
from contextlib import ExitStack

import numpy as np

import concourse.bacc as bacc
import concourse.bass as bass
import concourse.mybir as mybir
import concourse.tile as tile
from concourse.bass_utils import run_bass_kernel_spmd
from concourse.masks import make_identity

# Problem sizes (hardcoded per the harness contract).
B = 8
S = 2048          # S1 == S2
D = 512           # D1 == D2
F = 256           # fusion dim
N_CORES = 8
P = 128           # partitions

DC = D // P       # 4 d-chunks
FC = F // P       # 2 f-chunks
NS = S // P       # 16 s-tiles
SUPER = 512       # s1 super-block width for scores
NSUP = S // SUPER # 4 super-blocks

FP32 = mybir.dt.float32
BF16 = mybir.dt.bfloat16

# float32r streams through the PE at 1 cycle/row (vs 4 for plain fp32) when
# the moving free dim is >= 256. The BIR verifier requires every tensor a
# f32r matmul consumes to be *produced* rounded to f32r, so all matmul-facing
# SBUF tiles are allocated in this dtype and their producers cast on write.
MM_DT = mybir.dt.float32r
# Attention-stage storage dtype. bf16 weights enable Fast Weight Load (the
# fp32/f32r LDWEIGHTS is a serial ~213ns per matmul; bf16's is ~27ns and
# pipelined), and bf16 streams 1 cycle/row. PSUM accumulation stays fp32.
AT_DT = BF16


def attention_body(ctx, tc, out, feat1, feat2, Wq, bq, Wk, bk, Wv, bv, Wfc, bfc):
    """Emit the per-core attention program.

    out:   [S, F] fp32 DRAM
    feat1: [S, D], feat2: [S, D] fp32 DRAM
    Wq/Wk: [D, F], Wv: [D, F], Wfc: [F, F], biases [F] fp32 DRAM
    """
    nc = tc.nc
    Ident = mybir.ActivationFunctionType.Identity
    Exp = mybir.ActivationFunctionType.Exp
    scale = 1.0 / float(np.sqrt(F))

    # ---------------- constant / persistent pools ----------------
    consts = ctx.enter_context(tc.tile_pool(name="consts", bufs=1))
    persist = ctx.enter_context(tc.tile_pool(name="persist", bufs=1))

    # Persistent activations (allocated early: the ones-column memset must be
    # the first gpsimd work so the first PE instruction's single Pool wait
    # covers every gpsimd-produced constant).
    qt_sb = persist.tile([P, FC, S], AT_DT)   # Q^T  [f, s1]
    kt_sb = persist.tile([P, FC, S], AT_DT)   # K^T  [f, s2]
    # V padded to F+2 columns: col F is the softmax-denominator ones column;
    # col F+1 is dead padding (f32r matmuls need an even moving free dim).
    v_sb = persist.tile([P, NS, F + 2], AT_DT)  # V (+ ones col) [s2, f+2]
    # gpsimd memset can't write f32r, so stage in fp32 and cast-copy on DVE
    # (a legal f32r producer).
    ones_stage = consts.tile([P, NS, 2], FP32)
    nc.gpsimd.memset(ones_stage[:], 1.0)
    nc.vector.tensor_copy(v_sb[:, :, F:F + 2], ones_stage[:])

    ident = consts.tile([P, P], FP32)
    make_identity(nc, ident[:])
    ident_bf = consts.tile([P, P], BF16)
    nc.vector.tensor_copy(ident_bf[:], ident[:])

    # Weights, rearranged so the contraction dim is on partitions. DMA lands
    # fp32; a one-time engine copy rounds into the matmul dtype.
    def load_weight(W, shape, pattern, name, dt):
        stage = consts.tile(shape, FP32, tag=f"stage_{name}")
        nc.scalar.dma_start(stage[:], W.rearrange(pattern, p=P))
        if dt == FP32:
            return stage
        w = consts.tile(shape, dt, tag=f"w_{name}")
        nc.vector.tensor_copy(w[:], stage[:])
        return w

    wq_sb = load_weight(Wq, [P, DC, F], "(c p) f -> p c f", "wq", AT_DT)
    wk_sb = load_weight(Wk, [P, DC, F], "(c p) f -> p c f", "wk", AT_DT)
    wv_sb = load_weight(Wv, [P, DC, F], "(c p) f -> p c f", "wv", AT_DT)
    wfc_sb = load_weight(Wfc, [P, FC, F], "(c p) g -> p c g", "wfc", AT_DT)

    # Per-partition biases for the [f, s] layouts.
    bq_sb = consts.tile([P, FC], FP32)
    nc.scalar.dma_start(bq_sb[:], bq.rearrange("(c p) -> p c", p=P))
    bk_sb = consts.tile([P, FC], FP32)
    nc.scalar.dma_start(bk_sb[:], bk.rearrange("(c p) -> p c", p=P))
    # Free-dim broadcast biases for the natural layouts.
    bv_bc = consts.tile([P, F], FP32)
    nc.scalar.dma_start(bv_bc[:], bv.partition_broadcast(P))
    bfc_bc = consts.tile([P, F], FP32)
    nc.scalar.dma_start(bfc_bc[:], bfc.partition_broadcast(P))

    # ---------------- phase 1: transposes + projections ----------------
    with ExitStack() as ph1:
        featT_pool = ph1.enter_context(tc.tile_pool(name="featT", bufs=1))
        ld_pool = ph1.enter_context(tc.tile_pool(name="ld", bufs=6))
        ps_t = ph1.enter_context(tc.tile_pool(name="ps_t", bufs=4, space="PSUM"))
        ps_proj = ph1.enter_context(tc.tile_pool(name="ps_proj", bufs=2, space="PSUM"))
        ps_v = ph1.enter_context(tc.tile_pool(name="ps_v", bufs=2, space="PSUM"))

        f1T = featT_pool.tile([P, DC, S], AT_DT)  # feat1^T [d, s1]
        f2T = featT_pool.tile([P, DC, S], AT_DT)  # feat2^T [d, s2]

        feat1_r = feat1.rearrange("(n p) d -> p n d", p=P)  # [128, 16, 512]
        feat2_r = feat2.rearrange("(n p) d -> p n d", p=P)

        def load_pair(feat_r, pair):
            """DMA two 128-row s-tiles through SWDGE (gpsimd), which casts
            fp32->bf16 in flight. The Pool queue is otherwise idle so feat
            triggers never contend with ACT/DVE compute or out-DMAs."""
            ft = ld_pool.tile([P, 2, D], BF16, tag="ld")
            nc.gpsimd.dma_start(ft[:], feat_r[:, 2 * pair:2 * pair + 2, :])
            return ft

        def transpose_tile(ft_slice, fT, i):
            """PE-transpose a loaded s-tile's 4 d-chunks into fT.

            Transposes run as REGULAR bf16 matmuls (ft.T @ I): unlike
            transpose-mode (latency-bound ~250ns, no pipelining),
            back-to-back bf16 N=128 matmuls stream at ~81ns with FWL-hidden
            weight loads. All 4 land in one PSUM bank, drained by one DVE
            copy.
            """
            pst = ps_t.tile([P, D], FP32, tag="ps_t")
            for dc in range(DC):
                nc.tensor.matmul(
                    pst[:, dc * P:(dc + 1) * P], ft_slice[:, dc * P:(dc + 1) * P],
                    ident_bf[:], start=True, stop=True,
                )
            nc.vector.tensor_copy(
                fT[:, :, i * P:(i + 1) * P],
                pst[:].rearrange("p (c s) -> p c s", c=DC),
            )

        # Prefetch ring over PAIR loads: emission keeps a few pair-DMAs in
        # flight ahead of the transposes across the phase-1 schedule.
        schedule = []
        for sc in range(NSUP):
            schedule.append((feat1_r, f1T, 2 * sc))
            schedule.append((feat1_r, f1T, 2 * sc + 1))
            schedule.append((feat2_r, f2T, 2 * sc))
            schedule.append((feat2_r, f2T, 2 * sc + 1))
        loads = {}
        PREFETCH = 3
        for k in range(PREFETCH):
            fr, fT, pair = schedule[k]
            loads[k] = load_pair(fr, pair)
        emitted = PREFETCH

        def run_transpose_pair(k):
            nonlocal emitted
            fr, fT, pair = schedule[k]
            ft = loads.pop(k)
            for j in range(2):
                transpose_tile(ft[:, j], fT, 2 * pair + j)
            if emitted < len(schedule):
                fr2, _, pair2 = schedule[emitted]
                loads[emitted] = load_pair(fr2, pair2)
                emitted += 1

        tk = 0  # next schedule index to transpose
        for sc in range(NSUP):
            s_lo, s_hi = sc * SUPER, (sc + 1) * SUPER
            for _ in range(2):
                run_transpose_pair(tk); tk += 1
            # Q^T for this s1 super-block.
            for fc in range(FC):
                psq = ps_proj.tile([P, SUPER], FP32, tag="ps_proj")
                for dc in range(DC):
                    nc.tensor.matmul(
                        psq[:],
                        wq_sb[:, dc, fc * P:(fc + 1) * P],
                        f1T[:, dc, s_lo:s_hi],
                        start=(dc == 0), stop=(dc == DC - 1),
                    )
                nc.scalar.activation(
                    qt_sb[:, fc, s_lo:s_hi], psq[:], Ident,
                    bias=bq_sb[:, fc:fc + 1],
                )
            for _ in range(2):
                run_transpose_pair(tk); tk += 1
            # K^T for this s2 super-block.
            for fc in range(FC):
                psk = ps_proj.tile([P, SUPER], FP32, tag="ps_proj")
                for dc in range(DC):
                    nc.tensor.matmul(
                        psk[:],
                        wk_sb[:, dc, fc * P:(fc + 1) * P],
                        f2T[:, dc, s_lo:s_hi],
                        start=(dc == 0), stop=(dc == DC - 1),
                    )
                nc.scalar.activation(
                    kt_sb[:, fc, s_lo:s_hi], psk[:], Ident,
                    bias=bk_sb[:, fc:fc + 1],
                )
            # V (natural layout) for the 4 s2-tiles of this super-block.
            for i in range(sc * 4, sc * 4 + 4):
                psv = ps_v.tile([P, F], FP32, tag="ps_v")
                for dc in range(DC):
                    nc.tensor.matmul(
                        psv[:],
                        f2T[:, dc, i * P:(i + 1) * P],
                        wv_sb[:, dc, :],
                        start=(dc == 0), stop=(dc == DC - 1),
                    )
                nc.vector.tensor_add(v_sb[:, i, 0:F], psv[:], bv_bc[:])

    # ---------------- phase 2: attention ----------------
    with ExitStack() as ph2:
        pt_pool = ph2.enter_context(tc.tile_pool(name="pt", bufs=2))
        ao_pool = ph2.enter_context(tc.tile_pool(name="ao", bufs=3))
        ps_sc = ph2.enter_context(tc.tile_pool(name="ps_sc", bufs=2, space="PSUM"))
        ps_at = ph2.enter_context(tc.tile_pool(name="ps_at", bufs=2, space="PSUM"))
        ps_sm = ph2.enter_context(tc.tile_pool(name="ps_sm", bufs=2, space="PSUM"))

        def emit_score_group(sup, g, pt):
            """One scores^T group: s2-chunk pair (2g, 2g+1) accumulated into
            a 2-bank PSUM tile, exp'd (1024 cols) straight into pt."""
            s_lo, s_hi = sup * SUPER, (sup + 1) * SUPER
            s2c = 2 * g
            pss = ps_sc.tile([P, 2, SUPER], FP32, tag="ps_sc")
            for half in range(2):
                for fc in range(FC):
                    nc.tensor.matmul(
                        pss[:, half, :],
                        kt_sb[:, fc, (s2c + half) * P:(s2c + half + 1) * P],
                        qt_sb[:, fc, s_lo:s_hi],
                        start=(fc == 0), stop=(fc == FC - 1),
                    )
            nc.scalar.activation(pt[:, s2c:s2c + 2, :], pss[:], Exp, scale=scale)

        def emit_pv_block(sup, b, pt):
            """PV + normalize + attn_out^T transpose + final projection +
            store for one 128-row s1 block."""
            blk = sup * SUPER + b * P
            psa = ps_at.tile([P, F + 2], FP32, tag="ps_at")
            for s2c in range(NS):
                nc.tensor.matmul(
                    psa[:],
                    pt[:, s2c, b * P:(b + 1) * P],
                    v_sb[:, s2c, :],
                    start=(s2c == 0), stop=(s2c == NS - 1),
                )
            # Normalize by the softmax denominator (ones-column).
            recip = ao_pool.tile([P, 1], FP32, tag="recip")
            nc.vector.reciprocal(recip[:], psa[:, F:F + 1])
            ao = ao_pool.tile([P, F], AT_DT, tag="ao")
            nc.vector.tensor_scalar_mul(ao[:], psa[:, 0:F], recip[:])
            # attn_out^T via PE transpose for the final contraction over f.
            pst = ps_sm.tile([P, FC, P], FP32, tag="ps_sm")
            for fc in range(FC):
                nc.tensor.matmul(
                    pst[:, fc, :], ao[:, fc * P:(fc + 1) * P], ident_bf[:],
                    start=True, stop=True,
                )
            aot = ao_pool.tile([P, FC, P], AT_DT, tag="aot")
            nc.vector.tensor_copy(aot[:], pst[:])
            pso = ps_sm.tile([P, F], FP32, tag="ps_sm")
            for fc in range(FC):
                nc.tensor.matmul(
                    pso[:],
                    aot[:, fc, :],
                    wfc_sb[:, fc, :],
                    start=(fc == 0), stop=(fc == FC - 1),
                )
            o_sb = ao_pool.tile([P, F], FP32, tag="o_sb")
            nc.vector.tensor_add(o_sb[:], pso[:], bfc_bc[:])
            nc.sync.dma_start(out[blk:blk + P, :], o_sb[:])

        # Software pipeline with fine-grained interleave: the exp of a scores
        # group (ACT) outruns its 4 matmuls, so a pure scores stretch is
        # ACT-paced. Interleaving PV blocks of super-block `sup` between
        # score groups of `sup+1` keeps the PE streaming while ACT drains.
        pt_cur = pt_pool.tile([P, NS, SUPER], AT_DT, tag="pt")
        for g in range(NS // 2):
            emit_score_group(0, g, pt_cur)
        for sup in range(NSUP):
            pt = pt_cur
            if sup + 1 < NSUP:
                pt_cur = pt_pool.tile([P, NS, SUPER], AT_DT, tag="pt")
            for b in range(4):
                if sup + 1 < NSUP:
                    emit_score_group(sup + 1, 2 * b, pt_cur)
                    emit_score_group(sup + 1, 2 * b + 1, pt_cur)
                emit_pv_block(sup, b, pt)


def build_program():
    # Bacc (not raw Bass): its compile() legalizes semaphore waits to the
    # TRN2 one-wait-per-instruction constraint (move_matmul_waits_to_ldweights
    # + generate_event_semaphores), which walrus codegen requires.
    nc = bacc.Bacc("TRN2", target_bir_lowering=False, debug=False)
    feat1 = nc.dram_tensor("feat1", [S, D], FP32, kind="ExternalInput").ap()
    feat2 = nc.dram_tensor("feat2", [S, D], FP32, kind="ExternalInput").ap()
    Wq = nc.dram_tensor("Wq", [D, F], FP32, kind="ExternalInput").ap()
    bq = nc.dram_tensor("bq", [F], FP32, kind="ExternalInput").ap()
    Wk = nc.dram_tensor("Wk", [D, F], FP32, kind="ExternalInput").ap()
    bk = nc.dram_tensor("bk", [F], FP32, kind="ExternalInput").ap()
    Wv = nc.dram_tensor("Wv", [D, F], FP32, kind="ExternalInput").ap()
    bv = nc.dram_tensor("bv", [F], FP32, kind="ExternalInput").ap()
    Wfc = nc.dram_tensor("Wfc", [F, F], FP32, kind="ExternalInput").ap()
    bfc = nc.dram_tensor("bfc", [F], FP32, kind="ExternalInput").ap()
    out = nc.dram_tensor("out", [S, F], FP32, kind="ExternalOutput").ap()

    with tile.TileContext(nc) as tc, ExitStack() as ctx:
        attention_body(ctx, tc, out, feat1, feat2, Wq, bq, Wk, bk, Wv, bv, Wfc, bfc)
    nc.compile()
    return nc


def run(inputs, trace=False, trace_kwargs=None):
    """Shard over 8 cores, execute, gather. Returns (output, BassKernelResults)."""
    nc = build_program()
    shared = {
        k: np.ascontiguousarray(np.asarray(inputs[k], dtype=np.float32))
        for k in ("Wq", "bq", "Wk", "bk", "Wv", "bv", "Wfc", "bfc")
    }
    feat1 = np.asarray(inputs["feat1"], dtype=np.float32)
    feat2 = np.asarray(inputs["feat2"], dtype=np.float32)
    in_maps = [
        {
            "feat1": np.ascontiguousarray(feat1[i]),
            "feat2": np.ascontiguousarray(feat2[i]),
            **shared,
        }
        for i in range(N_CORES)
    ]
    res = run_bass_kernel_spmd(
        nc, in_maps, core_ids=list(range(N_CORES)),
        trace=trace, **(trace_kwargs or {}),
    )
    out = np.stack([res.results[i]["out"] for i in range(N_CORES)], axis=0)
    return out, res


def kernel(**inputs) -> np.ndarray:
    out, _ = run(inputs)
    return out



# revision 3
# speedup vs baseline: 1.0155x; 1.0155x over previous
"""Fused attention kernel for Trainium2, SPMD over 8 NeuronCores.

Problem: nn_AttentionFusion (B=8, S1=S2=2048, D1=D2=512, F=256, fp32).

    Q = feat1 @ Wq + bq            [B,S1,F]
    K = feat2 @ Wk + bk            [B,S2,F]
    V = feat2 @ Wv + bv            [B,S2,F]
    A = softmax(Q K^T / sqrt(F))   [B,S1,S2]
    out = (A @ V) @ Wfc + bfc      [B,S1,F]

Sharding: pure data-parallel over batch - core i computes batch element i.

Per-core algorithm (v2):
  *  Wfc is folded into the V projection: A@V@Wfc == A@(V@Wfc), so the kernel
     precomputes Wv' = Wv@Wfc on the PE during the initial DMA lead-in (when
     the PE would otherwise idle) and never materializes attn_out - each PV
     result block is normalized, biased and DMA'd straight out. The V bias
     folds into a constant output bias: obias = bv@Wfc + bfc (A rows sum to 1).
  *  feat2 is processed first (K^T in [f,s2], V' in [s2,g] with a ones column
     for the softmax denominator), then feat1 super-block 0 -> Q^T [f,s1].
     The main loop pipelines scores^T(sup) / f1+Q(sup+1) / PV(sup-1) so the
     exp drains (ACT) and DVE drains overlap the PE stream.
  *  scores^T = K^T-chunk.T @ Q^T with exp fused into the PSUM drain (no max
     subtraction: scores ~ N(0,1), fp32-safe). P^T lands exactly in the
     layout the PV matmul needs as stationary operand; (P@V')/denom with the
     denominator from the ones column; output = psa*recip + obias in one
     fused DVE op.
  *  DMA priority: wv/wfc first on the sync queue (they gate the Wv'
     precompute), feat2 pairs + wk/wq interleaved on the gpsimd SWDGE queue
     (which casts fp32->bf16 in flight) so the first feat tiles are not
     queued behind megabytes of weights.
"""

import os
from contextlib import ExitStack

import numpy as np

import concourse.bacc as bacc
import concourse.bass as bass
import concourse.mybir as mybir
import concourse.tile as tile
from concourse.bass_utils import run_bass_kernel_spmd
from concourse.masks import make_identity

# Problem sizes (hardcoded per the harness contract).
B = 8
S = 2048          # S1 == S2
D = 512           # D1 == D2
F = 256           # fusion dim (also the output dim G of Wfc)
G = 256
N_CORES = 8
P = 128           # partitions

DC = D // P       # 4 d-chunks
FC = F // P       # 2 f-chunks
NS = S // P       # 16 s-tiles
SUPER = 512       # s1 super-block width for scores
NSUP = S // SUPER # 4 super-blocks
PREFETCH = 5      # feat pair-loads kept in flight ahead of the PE

FP32 = mybir.dt.float32
BF16 = mybir.dt.bfloat16


def attention_body(ctx, tc, out, feat1, feat2, Wq, bq, Wk, bk, Wv, bv, Wfc, bfc):
    """Emit the per-core attention program.

    out:   [S, G] fp32 DRAM
    feat1: [S, D], feat2: [S, D] fp32 DRAM
    Wq/Wk/Wv: [D, F], Wfc: [F, G], biases [F]/[G] fp32 DRAM
    """
    nc = tc.nc
    Ident = mybir.ActivationFunctionType.Identity
    Exp = mybir.ActivationFunctionType.Exp
    Mult = mybir.AluOpType.mult
    Add = mybir.AluOpType.add
    scale = 1.0 / float(np.sqrt(F))

    # ---------------- pools ----------------
    consts = ctx.enter_context(tc.tile_pool(name="consts", bufs=1))
    persist = ctx.enter_context(tc.tile_pool(name="persist", bufs=1))
    ld_pool = ctx.enter_context(tc.tile_pool(name="ld", bufs=6))
    pt_pool = ctx.enter_context(tc.tile_pool(name="pt", bufs=2))
    ao_pool = ctx.enter_context(tc.tile_pool(name="ao", bufs=3))
    # PSUM: scores tiles are 2 banks x2 bufs; everything else 1 bank x4 bufs.
    ps_sc = ctx.enter_context(tc.tile_pool(name="ps_sc", bufs=2, space="PSUM"))
    ps_sm = ctx.enter_context(tc.tile_pool(name="ps_sm", bufs=4, space="PSUM"))

    # Persistent activations.
    qt_sb = persist.tile([P, FC, S], BF16)      # Q^T  [f, s1]
    kt_sb = persist.tile([P, FC, S], BF16)      # K^T  [f, s2]
    # V' = feat2 @ (Wv@Wfc), padded to G+2 cols: col G is the softmax
    # denominator ones column, col G+1 is dead padding.
    v2_sb = persist.tile([P, NS, G + 2], BF16)  # V' (+ones col) [s2, g+2]
    f1T = persist.tile([P, DC, S], BF16)        # feat1^T [d, s1]
    f2T = persist.tile([P, DC, S], BF16)        # feat2^T [d, s2]

    # gpsimd constants first so later engine waits are cheap. memset a
    # contiguous fp32 stage, DVE casts into the strided bf16 ones column.
    ones_stage = consts.tile([P, NS, 2], FP32)
    nc.gpsimd.memset(ones_stage[:], 1.0)
    nc.vector.tensor_copy(v2_sb[:, :, G:G + 2], ones_stage[:])
    ones128 = consts.tile([P, P], FP32)
    nc.gpsimd.memset(ones128[:], 1.0)

    ident = consts.tile([P, P], FP32)
    make_identity(nc, ident[:])
    ident_bf = consts.tile([P, P], BF16)
    nc.vector.tensor_copy(ident_bf[:], ident[:])

    # ---------------- DMA issue, in priority order ----------------
    # wv/wfc gate the PE's lead-in work (Wv' precompute): sync HWDGE queue,
    # descriptors enqueued immediately.
    def load_weight_sync(W, shape, pattern, name):
        stage = consts.tile(shape, FP32, tag=f"stage_{name}")
        nc.sync.dma_start(stage[:], W.rearrange(pattern, p=P))
        w = consts.tile(shape, BF16, tag=f"w_{name}")
        nc.vector.tensor_copy(w[:], stage[:])
        return w

    wv_sb = load_weight_sync(Wv, [P, DC, F], "(c p) f -> p c f", "wv")
    wfc_sb = load_weight_sync(Wfc, [P, FC, G], "(c p) g -> p c g", "wfc")

    # feat pair loads ride the gpsimd SWDGE queue (casts fp32->bf16 in
    # flight). wk/wq are interleaved into the same stream so they land after
    # the first feat2 pairs but before they are needed.
    feat1_r = feat1.rearrange("(n p) d -> p n d", p=P)  # [128, 16, 512]
    feat2_r = feat2.rearrange("(n p) d -> p n d", p=P)
    schedule = [(feat2_r, f2T, pair) for pair in range(NS // 2)] + \
               [(feat1_r, f1T, pair) for pair in range(NS // 2)]
    loads = {}

    def issue_load(k):
        feat_r, _, pair = schedule[k]
        ft = ld_pool.tile([P, 2, D], BF16, tag="ld")
        nc.gpsimd.dma_start(ft[:], feat_r[:, 2 * pair:2 * pair + 2, :])
        loads[k] = ft

    issue_load(0)
    issue_load(1)
    wk_sb = consts.tile([P, DC, F], BF16)
    nc.gpsimd.dma_start(wk_sb[:], Wk.rearrange("(c p) f -> p c f", p=P))
    issue_load(2)
    issue_load(3)
    wq_sb = consts.tile([P, DC, F], BF16)
    nc.gpsimd.dma_start(wq_sb[:], Wq.rearrange("(c p) f -> p c f", p=P))
    issue_load(4)
    emitted = PREFETCH

    # Biases (tiny) on the scalar queue.
    bq_sb = consts.tile([P, FC], FP32)
    nc.scalar.dma_start(bq_sb[:], bq.rearrange("(c p) -> p c", p=P))
    bk_sb = consts.tile([P, FC], FP32)
    nc.scalar.dma_start(bk_sb[:], bk.rearrange("(c p) -> p c", p=P))
    bv_part = consts.tile([P, FC], FP32)
    nc.scalar.dma_start(bv_part[:], bv.rearrange("(c p) -> p c", p=P))
    bfc_bc = consts.tile([P, G], FP32)
    nc.scalar.dma_start(bfc_bc[:], bfc.partition_broadcast(P))

    # ---------------- PE lead-in: Wv' = Wv@Wfc and obias ----------------
    # Runs while the first feat2 pairs are still in the DMA queues.
    wvT = consts.tile([P, FC, D], BF16)   # Wv^T [f, d]
    for fc in range(FC):
        pst = ps_sm.tile([P, D], FP32, tag="ps_sm")
        for dc in range(DC):
            nc.tensor.matmul(
                pst[:, dc * P:(dc + 1) * P],
                wv_sb[:, dc, fc * P:(fc + 1) * P], ident_bf[:],
                start=True, stop=True,
            )
        nc.vector.tensor_copy(wvT[:, fc, :], pst[:])
    wv2_sb = consts.tile([P, DC, G], BF16)  # Wv' [d, g]
    for dc in range(DC):
        psw = ps_sm.tile([P, G], FP32, tag="ps_sm")
        for fc in range(FC):
            nc.tensor.matmul(
                psw[:],
                wvT[:, fc, dc * P:(dc + 1) * P],
                wfc_sb[:, fc, :],
                start=(fc == 0), stop=(fc == FC - 1),
            )
        nc.vector.tensor_copy(wv2_sb[:, dc, :], psw[:])

    # obias = bv@Wfc + bfc, replicated on all partitions: stationary
    # Mb[:, fc, j] = bv[fc*128+p] (same value across j), so stat^T@wfc gives
    # every output partition the row bv@Wfc.
    Mb = consts.tile([P, FC, P], BF16)
    for fc in range(FC):
        nc.vector.tensor_scalar_mul(Mb[:, fc, :], ones128[:], bv_part[:, fc:fc + 1])
    ps_ob = ps_sm.tile([P, G], FP32, tag="ps_sm")
    for fc in range(FC):
        nc.tensor.matmul(
            ps_ob[:], Mb[:, fc, :], wfc_sb[:, fc, :],
            start=(fc == 0), stop=(fc == FC - 1),
        )
    obias_bc = consts.tile([P, G], FP32)
    nc.vector.tensor_add(obias_bc[:], ps_ob[:], bfc_bc[:])

    # ---------------- building blocks ----------------
    def run_transpose_pair(k):
        """Transpose one loaded pair (2 s-tiles x 4 d-chunks) into its
        featT tile via regular bf16 matmuls against the identity."""
        nonlocal emitted
        _, fT, pair = schedule[k]
        ft = loads.pop(k)
        for j in range(2):
            i = 2 * pair + j
            pst = ps_sm.tile([P, D], FP32, tag="ps_sm")
            for dc in range(DC):
                nc.tensor.matmul(
                    pst[:, dc * P:(dc + 1) * P], ft[:, j, dc * P:(dc + 1) * P],
                    ident_bf[:], start=True, stop=True,
                )
            nc.vector.tensor_copy(
                fT[:, :, i * P:(i + 1) * P],
                pst[:].rearrange("p (c s) -> p c s", c=DC),
            )
        if emitted < len(schedule):
            issue_load(emitted)
            emitted += 1

    def emit_proj(fT, w_sb, b_sb, dst, sup):
        """Q^T/K^T for one super-block: [f, s] = W-chunk.T @ featT."""
        s_lo, s_hi = sup * SUPER, (sup + 1) * SUPER
        for fc in range(FC):
            psq = ps_sm.tile([P, SUPER], FP32, tag="ps_sm")
            for dc in range(DC):
                nc.tensor.matmul(
                    psq[:],
                    w_sb[:, dc, fc * P:(fc + 1) * P],
                    fT[:, dc, s_lo:s_hi],
                    start=(dc == 0), stop=(dc == DC - 1),
                )
            nc.scalar.activation(
                dst[:, fc, s_lo:s_hi], psq[:], Ident, bias=b_sb[:, fc:fc + 1],
            )

    def emit_v2_tile(i):
        """V' tile i: [s2-128, g] = feat2T-chunk.T @ Wv' (ACT drain)."""
        psv = ps_sm.tile([P, G], FP32, tag="ps_sm")
        for dc in range(DC):
            nc.tensor.matmul(
                psv[:],
                f2T[:, dc, i * P:(i + 1) * P],
                wv2_sb[:, dc, :],
                start=(dc == 0), stop=(dc == DC - 1),
            )
        nc.scalar.activation(v2_sb[:, i, 0:G], psv[:], Ident)

    def emit_score_group(sup, g, pt):
        """One scores^T group: s2-chunk pair (2g, 2g+1) accumulated into a
        2-bank PSUM tile, exp'd (1024 cols) straight into pt."""
        s_lo, s_hi = sup * SUPER, (sup + 1) * SUPER
        s2c = 2 * g
        pss = ps_sc.tile([P, 2, SUPER], FP32, tag="ps_sc")
        for half in range(2):
            for fc in range(FC):
                nc.tensor.matmul(
                    pss[:, half, :],
                    kt_sb[:, fc, (s2c + half) * P:(s2c + half + 1) * P],
                    qt_sb[:, fc, s_lo:s_hi],
                    start=(fc == 0), stop=(fc == FC - 1),
                )
        nc.scalar.activation(pt[:, s2c:s2c + 2, :], pss[:], Exp, scale=scale)

    def emit_pv_block(sup, b, pt):
        """PV block: psa = P^T-chunks.T @ V'_aug; col G is the softmax
        denominator. out = psa*recip + obias in one fused DVE op, then DMA."""
        blk = sup * SUPER + b * P
        psa = ps_sm.tile([P, G + 2], FP32, tag="ps_sm")
        for s2c in range(NS):
            nc.tensor.matmul(
                psa[:],
                pt[:, s2c, b * P:(b + 1) * P],
                v2_sb[:, s2c, :],
                start=(s2c == 0), stop=(s2c == NS - 1),
            )
        recip = ao_pool.tile([P, 1], FP32, tag="recip")
        nc.vector.reciprocal_approx_fast(recip[:], psa[:, G:G + 1])
        o_sb = ao_pool.tile([P, G], FP32, tag="o_sb")
        nc.vector.scalar_tensor_tensor(
            o_sb[:], psa[:, 0:G], recip[:], obias_bc[:], Mult, Add,
        )
        nc.sync.dma_start(out[blk:blk + P, :], o_sb[:])

    # ---------------- phase A: feat2 -> K^T + V' ----------------
    for sup in range(NSUP):
        run_transpose_pair(2 * sup)
        run_transpose_pair(2 * sup + 1)
        emit_proj(f2T, wk_sb, bk_sb, kt_sb, sup)
        for i in range(4 * sup, 4 * sup + 4):
            emit_v2_tile(i)

    # ---------------- phase B: feat1 sup0 -> Q^T sup0 ----------------
    run_transpose_pair(8)
    run_transpose_pair(9)
    emit_proj(f1T, wq_sb, bq_sb, qt_sb, 0)

    # ---------------- phase C: pipelined scores / Q(sup+1) / PV(sup-1) ----
    pt_tiles = {}
    for sup in range(NSUP):
        pt_cur = pt_pool.tile([P, NS, SUPER], BF16, tag="pt")
        pt_tiles[sup] = pt_cur
        for g in range(NS // 2):
            emit_score_group(sup, g, pt_tiles[sup])
        if sup + 1 < NSUP:
            run_transpose_pair(10 + 2 * sup)
            run_transpose_pair(11 + 2 * sup)
            emit_proj(f1T, wq_sb, bq_sb, qt_sb, sup + 1)
        if sup >= 1:
            for b in range(4):
                emit_pv_block(sup - 1, b, pt_tiles[sup - 1])
    for b in range(4):
        emit_pv_block(NSUP - 1, b, pt_tiles[NSUP - 1])


def build_program():
    # Bacc (not raw Bass): its compile() legalizes semaphore waits to the
    # TRN2 one-wait-per-instruction constraint (move_matmul_waits_to_ldweights
    # + generate_event_semaphores), which walrus codegen requires.
    nc = bacc.Bacc("TRN2", target_bir_lowering=False, debug=False)
    feat1 = nc.dram_tensor("feat1", [S, D], FP32, kind="ExternalInput").ap()
    feat2 = nc.dram_tensor("feat2", [S, D], FP32, kind="ExternalInput").ap()
    Wq = nc.dram_tensor("Wq", [D, F], FP32, kind="ExternalInput").ap()
    bq = nc.dram_tensor("bq", [F], FP32, kind="ExternalInput").ap()
    Wk = nc.dram_tensor("Wk", [D, F], FP32, kind="ExternalInput").ap()
    bk = nc.dram_tensor("bk", [F], FP32, kind="ExternalInput").ap()
    Wv = nc.dram_tensor("Wv", [D, F], FP32, kind="ExternalInput").ap()
    bv = nc.dram_tensor("bv", [F], FP32, kind="ExternalInput").ap()
    Wfc = nc.dram_tensor("Wfc", [F, G], FP32, kind="ExternalInput").ap()
    bfc = nc.dram_tensor("bfc", [G], FP32, kind="ExternalInput").ap()
    out = nc.dram_tensor("out", [S, G], FP32, kind="ExternalOutput").ap()

    with tile.TileContext(nc) as tc, ExitStack() as ctx:
        attention_body(ctx, tc, out, feat1, feat2, Wq, bq, Wk, bk, Wv, bv, Wfc, bfc)
    nc.compile()
    return nc


def run(inputs, trace=False, trace_kwargs=None):
    """Shard over 8 cores, execute, gather. Returns (output, BassKernelResults)."""
    nc = build_program()
    shared = {
        k: np.ascontiguousarray(np.asarray(inputs[k], dtype=np.float32))
        for k in ("Wq", "bq", "Wk", "bk", "Wv", "bv", "Wfc", "bfc")
    }
    feat1 = np.asarray(inputs["feat1"], dtype=np.float32)
    feat2 = np.asarray(inputs["feat2"], dtype=np.float32)
    in_maps = [
        {
            "feat1": np.ascontiguousarray(feat1[i]),
            "feat2": np.ascontiguousarray(feat2[i]),
            **shared,
        }
        for i in range(N_CORES)
    ]
    res = run_bass_kernel_spmd(
        nc, in_maps, core_ids=list(range(N_CORES)),
        trace=trace, **(trace_kwargs or {}),
    )
    out = np.stack([res.results[i]["out"] for i in range(N_CORES)], axis=0)
    return out, res


def kernel(**inputs) -> np.ndarray:
    out, _ = run(inputs)
    return out


# revision 7
# speedup vs baseline: 1.0327x; 1.0169x over previous
"""Fused attention kernel for Trainium2, SPMD over 8 NeuronCores.

Problem: nn_AttentionFusion (B=8, S1=S2=2048, D1=D2=512, F=256, fp32).

    Q = feat1 @ Wq + bq            [B,S1,F]
    K = feat2 @ Wk + bk            [B,S2,F]
    V = feat2 @ Wv + bv            [B,S2,F]
    A = softmax(Q K^T / sqrt(F))   [B,S1,S2]
    out = (A @ V) @ Wfc + bfc      [B,S1,F]

Sharding: pure data-parallel over batch - core i computes batch element i.

Per-core algorithm (v2):
  *  Wfc is folded into the V projection: A@V@Wfc == A@(V@Wfc), so the kernel
     precomputes Wv' = Wv@Wfc on the PE during the initial DMA lead-in (when
     the PE would otherwise idle) and never materializes attn_out - each PV
     result block is normalized, biased and DMA'd straight out. The V bias
     folds into a constant output bias: obias = bv@Wfc + bfc (A rows sum to 1).
  *  feat2 is processed first (K^T in [f,s2], V' in [s2,g] with a ones column
     for the softmax denominator), then feat1 super-block 0 -> Q^T [f,s1].
     The main loop pipelines scores^T(sup) / f1+Q(sup+1) / PV(sup-1) so the
     exp drains (ACT) and DVE drains overlap the PE stream.
  *  scores^T = K^T-chunk.T @ Q^T with exp fused into the PSUM drain (no max
     subtraction: scores ~ N(0,1), fp32-safe). P^T lands exactly in the
     layout the PV matmul needs as stationary operand; (P@V')/denom with the
     denominator from the ones column; output = psa*recip + obias in one
     fused DVE op.
  *  DMA priority: wv/wfc first on the sync queue (they gate the Wv'
     precompute), feat2 pairs + wk/wq interleaved on the gpsimd SWDGE queue
     (which casts fp32->bf16 in flight) so the first feat tiles are not
     queued behind megabytes of weights.
"""

import os
from contextlib import ExitStack

import numpy as np

import concourse.bacc as bacc
import concourse.bass as bass
import concourse.mybir as mybir
import concourse.tile as tile
from concourse.bass_utils import run_bass_kernel_spmd
from concourse.masks import make_identity

# Problem sizes (hardcoded per the harness contract).
B = 8
S = 2048          # S1 == S2
D = 512           # D1 == D2
F = 256           # fusion dim (also the output dim G of Wfc)
G = 256
N_CORES = 8
P = 128           # partitions

DC = D // P       # 4 d-chunks
FC = F // P       # 2 f-chunks
NS = S // P       # 16 s-tiles
SUPER = 512       # s1 super-block width for scores
NSUP = S // SUPER # 4 super-blocks
PREFETCH = 8      # feat pair-loads kept in flight ahead of the PE

FP32 = mybir.dt.float32
BF16 = mybir.dt.bfloat16


def attention_body(ctx, tc, out, feat1, feat2, Wq, bq, Wk, bk, Wv, bv, Wfc, bfc):
    """Emit the per-core attention program.

    out:   [S, G] fp32 DRAM
    feat1: [S, D], feat2: [S, D] fp32 DRAM
    Wq/Wk/Wv: [D, F], Wfc: [F, G], biases [F]/[G] fp32 DRAM
    """
    nc = tc.nc
    Ident = mybir.ActivationFunctionType.Identity
    Exp = mybir.ActivationFunctionType.Exp
    Mult = mybir.AluOpType.mult
    Add = mybir.AluOpType.add
    scale = 1.0 / float(np.sqrt(F))

    # ---------------- pools ----------------
    consts = ctx.enter_context(tc.tile_pool(name="consts", bufs=1))
    persist = ctx.enter_context(tc.tile_pool(name="persist", bufs=1))
    ld_pool = ctx.enter_context(tc.tile_pool(name="ld", bufs=10))
    pt_pool = ctx.enter_context(tc.tile_pool(name="pt", bufs=2))
    ao_pool = ctx.enter_context(tc.tile_pool(name="ao", bufs=3))
    # PSUM: scores tiles are 2 banks x2 bufs; everything else 1 bank x4 bufs.
    ps_sc = ctx.enter_context(tc.tile_pool(name="ps_sc", bufs=2, space="PSUM"))
    ps_sm = ctx.enter_context(tc.tile_pool(name="ps_sm", bufs=4, space="PSUM"))

    # Persistent activations.
    qt_sb = persist.tile([P, FC, S], BF16)      # Q^T  [f, s1]
    kt_sb = persist.tile([P, FC, S], BF16)      # K^T  [f, s2]
    # V' = feat2 @ (Wv@Wfc), padded to G+2 cols: col G is the softmax
    # denominator ones column, col G+1 is dead padding.
    v2_sb = persist.tile([P, NS, G + 2], BF16)  # V' (+ones col) [s2, g+2]
    f1T = persist.tile([P, DC, S], BF16)        # feat1^T [d, s1]
    f2T = persist.tile([P, DC, S], BF16)        # feat2^T [d, s2]

    # gpsimd constants first so later engine waits are cheap. memset a
    # contiguous fp32 stage, DVE casts into the strided bf16 ones column.
    ones_stage = consts.tile([P, NS, 2], FP32)
    nc.gpsimd.memset(ones_stage[:], 1.0)
    nc.vector.tensor_copy(v2_sb[:, :, G:G + 2], ones_stage[:])
    ones128 = consts.tile([P, P], FP32)
    nc.gpsimd.memset(ones128[:], 1.0)

    ident = consts.tile([P, P], FP32)
    make_identity(nc, ident[:])
    ident_bf = consts.tile([P, P], BF16)
    nc.vector.tensor_copy(ident_bf[:], ident[:])

    # ---------------- DMA issue, in priority order ----------------
    # Everything sizable rides the gpsimd SWDGE queue (casts fp32->bf16 in
    # flight, no staging). wv/wfc go first - they gate the PE's lead-in work
    # (Wv' precompute); wk/wq are interleaved so they land after the first
    # feat2 pairs but before they are needed.
    wv_sb = consts.tile([P, DC, F], BF16)
    nc.gpsimd.dma_start(wv_sb[:], Wv.rearrange("(c p) f -> p c f", p=P))
    wfc_sb = consts.tile([P, FC, G], BF16)
    nc.gpsimd.dma_start(wfc_sb[:], Wfc.rearrange("(c p) g -> p c g", p=P))
    feat1_r = feat1.rearrange("(n p) d -> p n d", p=P)  # [128, 16, 512]
    feat2_r = feat2.rearrange("(n p) d -> p n d", p=P)
    schedule = [(feat2_r, f2T, pair) for pair in range(NS // 2)] + \
               [(feat1_r, f1T, pair) for pair in range(NS // 2)]
    loads = {}

    def issue_load(k):
        feat_r, _, pair = schedule[k]
        ft = ld_pool.tile([P, 2, D], BF16, tag="ld")
        nc.gpsimd.dma_start(ft[:], feat_r[:, 2 * pair:2 * pair + 2, :])
        loads[k] = ft

    issue_load(0)
    issue_load(1)
    wk_sb = consts.tile([P, DC, F], BF16)
    nc.gpsimd.dma_start(wk_sb[:], Wk.rearrange("(c p) f -> p c f", p=P))
    issue_load(2)
    issue_load(3)
    wq_sb = consts.tile([P, DC, F], BF16)
    nc.gpsimd.dma_start(wq_sb[:], Wq.rearrange("(c p) f -> p c f", p=P))
    for k in range(4, PREFETCH):
        issue_load(k)
    emitted = PREFETCH

    # Biases (tiny) on the scalar queue.
    bq_sb = consts.tile([P, FC], FP32)
    nc.scalar.dma_start(bq_sb[:], bq.rearrange("(c p) -> p c", p=P))
    bk_sb = consts.tile([P, FC], FP32)
    nc.scalar.dma_start(bk_sb[:], bk.rearrange("(c p) -> p c", p=P))
    bv_part = consts.tile([P, FC], FP32)
    nc.scalar.dma_start(bv_part[:], bv.rearrange("(c p) -> p c", p=P))
    bfc_bc = consts.tile([P, G], FP32)
    nc.scalar.dma_start(bfc_bc[:], bfc.partition_broadcast(P))

    # ---------------- PE lead-in: Wv' = Wv@Wfc and obias ----------------
    # Runs while the first feat2 pairs are still in the DMA queues.
    wvT = consts.tile([P, FC, D], BF16)   # Wv^T [f, d]
    for fc in range(FC):
        pst = ps_sm.tile([P, D], FP32, tag="ps_sm")
        for dc in range(DC):
            nc.tensor.matmul(
                pst[:, dc * P:(dc + 1) * P],
                wv_sb[:, dc, fc * P:(fc + 1) * P], ident_bf[:],
                start=True, stop=True,
            )
        nc.vector.tensor_copy(wvT[:, fc, :], pst[:])
    wv2_sb = consts.tile([P, DC, G], BF16)  # Wv' [d, g]
    for dc in range(DC):
        psw = ps_sm.tile([P, G], FP32, tag="ps_sm")
        for fc in range(FC):
            nc.tensor.matmul(
                psw[:],
                wvT[:, fc, dc * P:(dc + 1) * P],
                wfc_sb[:, fc, :],
                start=(fc == 0), stop=(fc == FC - 1),
            )
        nc.vector.tensor_copy(wv2_sb[:, dc, :], psw[:])

    # obias = bv@Wfc + bfc, replicated on all partitions: stationary
    # Mb[:, fc, j] = bv[fc*128+p] (same value across j), so stat^T@wfc gives
    # every output partition the row bv@Wfc.
    Mb = consts.tile([P, FC, P], BF16)
    for fc in range(FC):
        nc.vector.tensor_scalar_mul(Mb[:, fc, :], ones128[:], bv_part[:, fc:fc + 1])
    ps_ob = ps_sm.tile([P, G], FP32, tag="ps_sm")
    for fc in range(FC):
        nc.tensor.matmul(
            ps_ob[:], Mb[:, fc, :], wfc_sb[:, fc, :],
            start=(fc == 0), stop=(fc == FC - 1),
        )
    obias_bc = consts.tile([P, G], FP32)
    nc.vector.tensor_add(obias_bc[:], ps_ob[:], bfc_bc[:])

    # ---------------- building blocks ----------------
    def run_transpose_pair(k):
        """Transpose one loaded pair (2 s-tiles x 4 d-chunks) into its
        featT tile via regular bf16 matmuls against the identity."""
        nonlocal emitted
        _, fT, pair = schedule[k]
        ft = loads.pop(k)
        for j in range(2):
            i = 2 * pair + j
            pst = ps_sm.tile([P, D], FP32, tag="ps_sm")
            for dc in range(DC):
                nc.tensor.matmul(
                    pst[:, dc * P:(dc + 1) * P], ft[:, j, dc * P:(dc + 1) * P],
                    ident_bf[:], start=True, stop=True,
                )
            nc.vector.tensor_copy(
                fT[:, :, i * P:(i + 1) * P],
                pst[:].rearrange("p (c s) -> p c s", c=DC),
            )
        if emitted < len(schedule):
            issue_load(emitted)
            emitted += 1

    def emit_proj(fT, w_sb, b_sb, dst, sup):
        """Q^T/K^T for one super-block: [f, s] = W-chunk.T @ featT."""
        s_lo, s_hi = sup * SUPER, (sup + 1) * SUPER
        for fc in range(FC):
            psq = ps_sm.tile([P, SUPER], FP32, tag="ps_sm")
            for dc in range(DC):
                nc.tensor.matmul(
                    psq[:],
                    w_sb[:, dc, fc * P:(fc + 1) * P],
                    fT[:, dc, s_lo:s_hi],
                    start=(dc == 0), stop=(dc == DC - 1),
                )
            nc.scalar.activation(
                dst[:, fc, s_lo:s_hi], psq[:], Ident, bias=b_sb[:, fc:fc + 1],
            )

    def emit_v2_tile(i):
        """V' tile i: [s2-128, g] = feat2T-chunk.T @ Wv' (ACT drain)."""
        psv = ps_sm.tile([P, G], FP32, tag="ps_sm")
        for dc in range(DC):
            nc.tensor.matmul(
                psv[:],
                f2T[:, dc, i * P:(i + 1) * P],
                wv2_sb[:, dc, :],
                start=(dc == 0), stop=(dc == DC - 1),
            )
        nc.scalar.activation(v2_sb[:, i, 0:G], psv[:], Ident)

    def emit_score_group(sup, g, pt):
        """One scores^T group: s2-chunk pair (2g, 2g+1) accumulated into a
        2-bank PSUM tile, exp'd (1024 cols) straight into pt."""
        s_lo, s_hi = sup * SUPER, (sup + 1) * SUPER
        s2c = 2 * g
        pss = ps_sc.tile([P, 2, SUPER], FP32, tag="ps_sc")
        for half in range(2):
            for fc in range(FC):
                nc.tensor.matmul(
                    pss[:, half, :],
                    kt_sb[:, fc, (s2c + half) * P:(s2c + half + 1) * P],
                    qt_sb[:, fc, s_lo:s_hi],
                    start=(fc == 0), stop=(fc == FC - 1),
                )
        nc.scalar.activation(pt[:, s2c:s2c + 2, :], pss[:], Exp, scale=scale)

    def emit_pv_block(sup, b, pt):
        """PV block: psa = P^T-chunks.T @ V'_aug; col G is the softmax
        denominator. out = psa*recip + obias in one fused DVE op, then DMA."""
        blk = sup * SUPER + b * P
        psa = ps_sm.tile([P, G + 2], FP32, tag="ps_sm")
        for s2c in range(NS):
            nc.tensor.matmul(
                psa[:],
                pt[:, s2c, b * P:(b + 1) * P],
                v2_sb[:, s2c, :],
                start=(s2c == 0), stop=(s2c == NS - 1),
            )
        recip = ao_pool.tile([P, 1], FP32, tag="recip")
        nc.vector.reciprocal_approx_fast(recip[:], psa[:, G:G + 1])
        o_sb = ao_pool.tile([P, G], FP32, tag="o_sb")
        nc.vector.scalar_tensor_tensor(
            o_sb[:], psa[:, 0:G], recip[:], obias_bc[:], Mult, Add,
        )
        nc.sync.dma_start(out[blk:blk + P, :], o_sb[:])

    # ---------------- phase A: feat2 -> K^T + V' ----------------
    for sup in range(NSUP):
        run_transpose_pair(2 * sup)
        run_transpose_pair(2 * sup + 1)
        emit_proj(f2T, wk_sb, bk_sb, kt_sb, sup)
        for i in range(4 * sup, 4 * sup + 4):
            emit_v2_tile(i)

    # ---------------- phase B: feat1 sup0 -> Q^T sup0 ----------------
    run_transpose_pair(8)
    run_transpose_pair(9)
    emit_proj(f1T, wq_sb, bq_sb, qt_sb, 0)

    # ---------------- phase C: pipelined scores / Q(sup+1) / PV(sup-1) ----
    pt_tiles = {}
    for sup in range(NSUP):
        pt_cur = pt_pool.tile([P, NS, SUPER], BF16, tag="pt")
        pt_tiles[sup] = pt_cur
        for g in range(NS // 2):
            emit_score_group(sup, g, pt_tiles[sup])
        if sup + 1 < NSUP:
            run_transpose_pair(10 + 2 * sup)
            run_transpose_pair(11 + 2 * sup)
            emit_proj(f1T, wq_sb, bq_sb, qt_sb, sup + 1)
        if sup >= 1:
            for b in range(4):
                emit_pv_block(sup - 1, b, pt_tiles[sup - 1])
    for b in range(4):
        emit_pv_block(NSUP - 1, b, pt_tiles[NSUP - 1])


def build_program():
    # Bacc (not raw Bass): its compile() legalizes semaphore waits to the
    # TRN2 one-wait-per-instruction constraint (move_matmul_waits_to_ldweights
    # + generate_event_semaphores), which walrus codegen requires.
    nc = bacc.Bacc("TRN2", target_bir_lowering=False, debug=False)
    feat1 = nc.dram_tensor("feat1", [S, D], FP32, kind="ExternalInput").ap()
    feat2 = nc.dram_tensor("feat2", [S, D], FP32, kind="ExternalInput").ap()
    Wq = nc.dram_tensor("Wq", [D, F], FP32, kind="ExternalInput").ap()
    bq = nc.dram_tensor("bq", [F], FP32, kind="ExternalInput").ap()
    Wk = nc.dram_tensor("Wk", [D, F], FP32, kind="ExternalInput").ap()
    bk = nc.dram_tensor("bk", [F], FP32, kind="ExternalInput").ap()
    Wv = nc.dram_tensor("Wv", [D, F], FP32, kind="ExternalInput").ap()
    bv = nc.dram_tensor("bv", [F], FP32, kind="ExternalInput").ap()
    Wfc = nc.dram_tensor("Wfc", [F, G], FP32, kind="ExternalInput").ap()
    bfc = nc.dram_tensor("bfc", [G], FP32, kind="ExternalInput").ap()
    out = nc.dram_tensor("out", [S, G], FP32, kind="ExternalOutput").ap()

    with tile.TileContext(nc) as tc, ExitStack() as ctx:
        attention_body(ctx, tc, out, feat1, feat2, Wq, bq, Wk, bk, Wv, bv, Wfc, bfc)
    nc.compile()
    return nc


def run(inputs, trace=False, trace_kwargs=None):
    """Shard over 8 cores, execute, gather. Returns (output, BassKernelResults)."""
    nc = build_program()
    shared = {
        k: np.ascontiguousarray(np.asarray(inputs[k], dtype=np.float32))
        for k in ("Wq", "bq", "Wk", "bk", "Wv", "bv", "Wfc", "bfc")
    }
    feat1 = np.asarray(inputs["feat1"], dtype=np.float32)
    feat2 = np.asarray(inputs["feat2"], dtype=np.float32)
    in_maps = [
        {
            "feat1": np.ascontiguousarray(feat1[i]),
            "feat2": np.ascontiguousarray(feat2[i]),
            **shared,
        }
        for i in range(N_CORES)
    ]
    res = run_bass_kernel_spmd(
        nc, in_maps, core_ids=list(range(N_CORES)),
        trace=trace, **(trace_kwargs or {}),
    )
    out = np.stack([res.results[i]["out"] for i in range(N_CORES)], axis=0)
    return out, res


def kernel(**inputs) -> np.ndarray:
    out, _ = run(inputs)
    return out


# revision 13
# speedup vs baseline: 1.0572x; 1.0238x over previous
"""Fused attention kernel for Trainium2, SPMD over 8 NeuronCores.

Problem: nn_AttentionFusion (B=8, S1=S2=2048, D1=D2=512, F=256, fp32).

    Q = feat1 @ Wq + bq            [B,S1,F]
    K = feat2 @ Wk + bk            [B,S2,F]
    V = feat2 @ Wv + bv            [B,S2,F]
    A = softmax(Q K^T / sqrt(F))   [B,S1,S2]
    out = (A @ V) @ Wfc + bfc      [B,S1,F]

Sharding: pure data-parallel over batch - core i computes batch element i.

Per-core algorithm (v2):
  *  Wfc is folded into the V projection: A@V@Wfc == A@(V@Wfc), so the kernel
     precomputes Wv' = Wv@Wfc on the PE during the initial DMA lead-in (when
     the PE would otherwise idle) and never materializes attn_out - each PV
     result block is normalized, biased and DMA'd straight out. The V bias
     folds into a constant output bias: obias = bv@Wfc + bfc (A rows sum to 1).
  *  feat2 is processed first (K^T in [f,s2], V' in [s2,g] with a ones column
     for the softmax denominator), then feat1 super-block 0 -> Q^T [f,s1].
     The main loop pipelines scores^T(sup) / f1+Q(sup+1) / PV(sup-1) so the
     exp drains (ACT) and DVE drains overlap the PE stream.
  *  scores^T = K^T-chunk.T @ Q^T with exp fused into the PSUM drain (no max
     subtraction: scores ~ N(0,1), fp32-safe). P^T lands exactly in the
     layout the PV matmul needs as stationary operand; (P@V')/denom with the
     denominator from the ones column; output = psa*recip + obias in one
     fused DVE op.
  *  DMA priority: wv/wfc first on the sync queue (they gate the Wv'
     precompute), feat2 pairs + wk/wq interleaved on the gpsimd SWDGE queue
     (which casts fp32->bf16 in flight) so the first feat tiles are not
     queued behind megabytes of weights.
"""

import os
from contextlib import ExitStack

import numpy as np

import concourse.bacc as bacc
import concourse.bass as bass
import concourse.mybir as mybir
import concourse.tile as tile
from concourse.bass_utils import run_bass_kernel_spmd
from concourse.masks import make_identity

# Problem sizes (hardcoded per the harness contract).
B = 8
S = 2048          # S1 == S2
D = 512           # D1 == D2
F = 256           # fusion dim (also the output dim G of Wfc)
G = 256
N_CORES = 8
P = 128           # partitions

DC = D // P       # 4 d-chunks
FC = F // P       # 2 f-chunks
NS = S // P       # 16 s-tiles
SUPER = 512       # s1 super-block width for scores
NSUP = S // SUPER # 4 super-blocks
PREFETCH = 4      # feat pair-loads kept in flight ahead of the PE. Keep this
                  # SHALLOW: the DMA queues serve in-flight transfers
                  # round-robin at descriptor granularity, so every extra
                  # in-flight load dilutes the bandwidth of the one the PE
                  # needs next.

FP32 = mybir.dt.float32
BF16 = mybir.dt.bfloat16


def attention_body(ctx, tc, out, feat1, feat2, Wq, bq, Wk, bk, Wv, bv, Wfc, bfc):
    """Emit the per-core attention program.

    out:   [S, G] fp32 DRAM
    feat1: [S, D], feat2: [S, D] fp32 DRAM
    Wq/Wk/Wv: [D, F], Wfc: [F, G], biases [F]/[G] fp32 DRAM
    """
    nc = tc.nc
    Ident = mybir.ActivationFunctionType.Identity
    Exp = mybir.ActivationFunctionType.Exp
    Mult = mybir.AluOpType.mult
    Add = mybir.AluOpType.add
    scale = 1.0 / float(np.sqrt(F))

    # ---------------- pools ----------------
    consts = ctx.enter_context(tc.tile_pool(name="consts", bufs=1))
    persist = ctx.enter_context(tc.tile_pool(name="persist", bufs=1))
    ld_pool = ctx.enter_context(tc.tile_pool(name="ld", bufs=6))
    pt_pool = ctx.enter_context(tc.tile_pool(name="pt", bufs=4))
    ao_pool = ctx.enter_context(tc.tile_pool(name="ao", bufs=3))
    # PSUM: scores tiles are 2 banks x2 bufs; everything else 1 bank x4 bufs.
    ps_sc = ctx.enter_context(tc.tile_pool(name="ps_sc", bufs=2, space="PSUM"))
    ps_sm = ctx.enter_context(tc.tile_pool(name="ps_sm", bufs=4, space="PSUM"))

    # Persistent activations.
    qt_sb = persist.tile([P, FC, S], BF16)      # Q^T  [f, s1]
    kt_sb = persist.tile([P, FC, S], BF16)      # K^T  [f, s2]
    # V' = feat2 @ (Wv@Wfc), padded to G+2 cols: col G is the softmax
    # denominator ones column, col G+1 is dead padding.
    v2_sb = persist.tile([P, NS, G + 2], BF16)  # V' (+ones col) [s2, g+2]
    f1T = persist.tile([P, DC, S], BF16)        # feat1^T [d, s1]
    f2T = persist.tile([P, DC, S], BF16)        # feat2^T [d, s2]

    # gpsimd constants first so later engine waits are cheap. memset a
    # contiguous fp32 stage, DVE casts into the strided bf16 ones column.
    ones_stage = consts.tile([P, NS, 2], FP32)
    nc.gpsimd.memset(ones_stage[:], 1.0)
    nc.vector.tensor_copy(v2_sb[:, :, G:G + 2], ones_stage[:])
    ones128 = consts.tile([P, P], FP32)
    nc.gpsimd.memset(ones128[:], 1.0)

    ident = consts.tile([P, P], FP32)
    make_identity(nc, ident[:])
    ident_bf = consts.tile([P, P], BF16)
    nc.vector.tensor_copy(ident_bf[:], ident[:])

    # ---------------- DMA issue, in priority order ----------------
    # Everything sizable rides the gpsimd SWDGE queue (casts fp32->bf16 in
    # flight, no staging). wfc/wv go first - they gate the PE's lead-in work
    # (Wv' precompute); wk/wq are interleaved so they land after the first
    # feat pairs but before they are needed.
    wfc_sb = consts.tile([P, FC, G], BF16)
    nc.gpsimd.dma_start(wfc_sb[:], Wfc.rearrange("(c p) g -> p c g", p=P))
    wv_sb = consts.tile([P, DC, F], BF16)
    nc.gpsimd.dma_start(wv_sb[:], Wv.rearrange("(c p) f -> p c f", p=P))
    # Consumption-ordered schedule: block s consumes f2 pairs 2s,2s+1 (for
    # K(s)+V'(s)) then f1 pairs 2s,2s+1 (for Q(s)).
    feat1_r = feat1.rearrange("(n p) d -> p n d", p=P)  # [128, 16, 512]
    feat2_r = feat2.rearrange("(n p) d -> p n d", p=P)
    schedule = []
    for s in range(NSUP):
        schedule.append((feat2_r, f2T, 2 * s))
        schedule.append((feat2_r, f2T, 2 * s + 1))
        schedule.append((feat1_r, f1T, 2 * s))
        schedule.append((feat1_r, f1T, 2 * s + 1))
    loads = {}

    def issue_load(k):
        feat_r, _, pair = schedule[k]
        ft = ld_pool.tile([P, 2, D], BF16, tag="ld")
        nc.gpsimd.dma_start(ft[:], feat_r[:, 2 * pair:2 * pair + 2, :])
        loads[k] = ft

    issue_load(0)
    issue_load(1)
    wk_sb = consts.tile([P, DC, F], BF16)
    nc.gpsimd.dma_start(wk_sb[:], Wk.rearrange("(c p) f -> p c f", p=P))
    issue_load(2)
    issue_load(3)
    wq_sb = consts.tile([P, DC, F], BF16)
    nc.gpsimd.dma_start(wq_sb[:], Wq.rearrange("(c p) f -> p c f", p=P))
    emitted = PREFETCH

    # Biases (tiny) on the scalar queue.
    bq_sb = consts.tile([P, FC], FP32)
    nc.scalar.dma_start(bq_sb[:], bq.rearrange("(c p) -> p c", p=P))
    bk_sb = consts.tile([P, FC], FP32)
    nc.scalar.dma_start(bk_sb[:], bk.rearrange("(c p) -> p c", p=P))
    bv_part = consts.tile([P, FC], FP32)
    nc.scalar.dma_start(bv_part[:], bv.rearrange("(c p) -> p c", p=P))
    bfc_bc = consts.tile([P, G], FP32)
    nc.scalar.dma_start(bfc_bc[:], bfc.partition_broadcast(P))

    # ---------------- PE lead-in: Wv' = Wv@Wfc and obias ----------------
    # Runs while the first feat2 pairs are still in the DMA queues.
    wvT = consts.tile([P, FC, D], BF16)   # Wv^T [f, d]
    for fc in range(FC):
        pst = ps_sm.tile([P, D], FP32, tag="ps_sm")
        for dc in range(DC):
            nc.tensor.matmul(
                pst[:, dc * P:(dc + 1) * P],
                wv_sb[:, dc, fc * P:(fc + 1) * P], ident_bf[:],
                start=True, stop=True,
            )
        nc.vector.tensor_copy(wvT[:, fc, :], pst[:])
    wv2_sb = consts.tile([P, DC, G], BF16)  # Wv' [d, g]
    for dc in range(DC):
        psw = ps_sm.tile([P, G], FP32, tag="ps_sm")
        for fc in range(FC):
            nc.tensor.matmul(
                psw[:],
                wvT[:, fc, dc * P:(dc + 1) * P],
                wfc_sb[:, fc, :],
                start=(fc == 0), stop=(fc == FC - 1),
            )
        nc.vector.tensor_copy(wv2_sb[:, dc, :], psw[:])

    # obias = bv@Wfc + bfc, replicated on all partitions: stationary
    # Mb[:, fc, j] = bv[fc*128+p] (same value across j), so stat^T@wfc gives
    # every output partition the row bv@Wfc.
    Mb = consts.tile([P, FC, P], BF16)
    for fc in range(FC):
        nc.vector.tensor_scalar_mul(Mb[:, fc, :], ones128[:], bv_part[:, fc:fc + 1])
    ps_ob = ps_sm.tile([P, G], FP32, tag="ps_sm")
    for fc in range(FC):
        nc.tensor.matmul(
            ps_ob[:], Mb[:, fc, :], wfc_sb[:, fc, :],
            start=(fc == 0), stop=(fc == FC - 1),
        )
    obias_bc = consts.tile([P, G], FP32)
    nc.vector.tensor_add(obias_bc[:], ps_ob[:], bfc_bc[:])

    # ---------------- building blocks ----------------
    def run_transpose_pair(k):
        """Transpose one loaded pair (2 s-tiles x 4 d-chunks) into its
        featT tile via regular bf16 matmuls against the identity."""
        nonlocal emitted
        _, fT, pair = schedule[k]
        ft = loads.pop(k)
        for j in range(2):
            i = 2 * pair + j
            pst = ps_sm.tile([P, D], FP32, tag="ps_sm")
            for dc in range(DC):
                nc.tensor.matmul(
                    pst[:, dc * P:(dc + 1) * P], ft[:, j, dc * P:(dc + 1) * P],
                    ident_bf[:], start=True, stop=True,
                )
            nc.vector.tensor_copy(
                fT[:, :, i * P:(i + 1) * P],
                pst[:].rearrange("p (c s) -> p c s", c=DC),
            )
        if emitted < len(schedule):
            issue_load(emitted)
            emitted += 1

    def emit_proj(fT, w_sb, b_sb, dst, sup):
        """Q^T/K^T for one super-block: [f, s] = W-chunk.T @ featT."""
        s_lo, s_hi = sup * SUPER, (sup + 1) * SUPER
        for fc in range(FC):
            psq = ps_sm.tile([P, SUPER], FP32, tag="ps_sm")
            for dc in range(DC):
                nc.tensor.matmul(
                    psq[:],
                    w_sb[:, dc, fc * P:(fc + 1) * P],
                    fT[:, dc, s_lo:s_hi],
                    start=(dc == 0), stop=(dc == DC - 1),
                )
            nc.scalar.activation(
                dst[:, fc, s_lo:s_hi], psq[:], Ident, bias=b_sb[:, fc:fc + 1],
            )

    def emit_v2_tile(i):
        """V' tile i: [s2-128, g] = feat2T-chunk.T @ Wv' (ACT drain)."""
        psv = ps_sm.tile([P, G], FP32, tag="ps_sm")
        for dc in range(DC):
            nc.tensor.matmul(
                psv[:],
                f2T[:, dc, i * P:(i + 1) * P],
                wv2_sb[:, dc, :],
                start=(dc == 0), stop=(dc == DC - 1),
            )
        nc.scalar.activation(v2_sb[:, i, 0:G], psv[:], Ident)

    def emit_score_group(sup, g, pt):
        """One scores^T group: s2-chunk pair (2g, 2g+1) accumulated into a
        2-bank PSUM tile, exp'd (1024 cols) straight into pt."""
        s_lo, s_hi = sup * SUPER, (sup + 1) * SUPER
        s2c = 2 * g
        pss = ps_sc.tile([P, 2, SUPER], FP32, tag="ps_sc")
        for half in range(2):
            for fc in range(FC):
                nc.tensor.matmul(
                    pss[:, half, :],
                    kt_sb[:, fc, (s2c + half) * P:(s2c + half + 1) * P],
                    qt_sb[:, fc, s_lo:s_hi],
                    start=(fc == 0), stop=(fc == FC - 1),
                )
        nc.scalar.activation(pt[:, s2c:s2c + 2, :], pss[:], Exp, scale=scale)

    def emit_pv_block(sup, b, pt):
        """PV block: psa = P^T-chunks.T @ V'_aug; col G is the softmax
        denominator. out = psa*recip + obias in one fused DVE op, then DMA."""
        blk = sup * SUPER + b * P
        psa = ps_sm.tile([P, G + 2], FP32, tag="ps_sm")
        for s2c in range(NS):
            nc.tensor.matmul(
                psa[:],
                pt[:, s2c, b * P:(b + 1) * P],
                v2_sb[:, s2c, :],
                start=(s2c == 0), stop=(s2c == NS - 1),
            )
        recip = ao_pool.tile([P, 1], FP32, tag="recip")
        nc.vector.reciprocal_approx_fast(recip[:], psa[:, G:G + 1])
        o_sb = ao_pool.tile([P, G], FP32, tag="o_sb")
        nc.vector.scalar_tensor_tensor(
            o_sb[:], psa[:, 0:G], recip[:], obias_bc[:], Mult, Add,
        )
        nc.sync.dma_start(out[blk:blk + P, :], o_sb[:])

    # ---------------- main: demand-ordered blocks ----------------
    # Block s (as its 4 feat pairs arrive): K(s) + V'(s) + Q(s), then every
    # score group the new K columns unlock - (q<s, g=2s/2s+1) against older
    # Q supers plus (s, g<=2s+1). This keeps PE work unlocked per arrived
    # byte from the first block, instead of serializing proj->scores phases.
    pt_tiles = {}
    for s in range(NSUP):
        run_transpose_pair(4 * s)          # f2 pair 2s
        run_transpose_pair(4 * s + 1)      # f2 pair 2s+1
        emit_proj(f2T, wk_sb, bk_sb, kt_sb, s)
        for i in range(4 * s, 4 * s + 4):
            emit_v2_tile(i)
        run_transpose_pair(4 * s + 2)      # f1 pair 2s
        run_transpose_pair(4 * s + 3)      # f1 pair 2s+1
        emit_proj(f1T, wq_sb, bq_sb, qt_sb, s)
        pt_cur = pt_pool.tile([P, NS, SUPER], BF16, tag="pt")
        pt_tiles[s] = pt_cur
        for q in range(s):
            emit_score_group(q, 2 * s, pt_tiles[q])
            emit_score_group(q, 2 * s + 1, pt_tiles[q])
        for g in range(2 * s + 2):
            emit_score_group(s, g, pt_tiles[s])

    # ---------------- PV + output ----------------
    for q in range(NSUP):
        for b in range(4):
            emit_pv_block(q, b, pt_tiles[q])


def build_program():
    # Bacc (not raw Bass): its compile() legalizes semaphore waits to the
    # TRN2 one-wait-per-instruction constraint (move_matmul_waits_to_ldweights
    # + generate_event_semaphores), which walrus codegen requires.
    nc = bacc.Bacc("TRN2", target_bir_lowering=False, debug=False)
    feat1 = nc.dram_tensor("feat1", [S, D], FP32, kind="ExternalInput").ap()
    feat2 = nc.dram_tensor("feat2", [S, D], FP32, kind="ExternalInput").ap()
    Wq = nc.dram_tensor("Wq", [D, F], FP32, kind="ExternalInput").ap()
    bq = nc.dram_tensor("bq", [F], FP32, kind="ExternalInput").ap()
    Wk = nc.dram_tensor("Wk", [D, F], FP32, kind="ExternalInput").ap()
    bk = nc.dram_tensor("bk", [F], FP32, kind="ExternalInput").ap()
    Wv = nc.dram_tensor("Wv", [D, F], FP32, kind="ExternalInput").ap()
    bv = nc.dram_tensor("bv", [F], FP32, kind="ExternalInput").ap()
    Wfc = nc.dram_tensor("Wfc", [F, G], FP32, kind="ExternalInput").ap()
    bfc = nc.dram_tensor("bfc", [G], FP32, kind="ExternalInput").ap()
    out = nc.dram_tensor("out", [S, G], FP32, kind="ExternalOutput").ap()

    with tile.TileContext(nc) as tc, ExitStack() as ctx:
        attention_body(ctx, tc, out, feat1, feat2, Wq, bq, Wk, bk, Wv, bv, Wfc, bfc)
    nc.compile()
    return nc


def run(inputs, trace=False, trace_kwargs=None):
    """Shard over 8 cores, execute, gather. Returns (output, BassKernelResults)."""
    nc = build_program()
    shared = {
        k: np.ascontiguousarray(np.asarray(inputs[k], dtype=np.float32))
        for k in ("Wq", "bq", "Wk", "bk", "Wv", "bv", "Wfc", "bfc")
    }
    feat1 = np.asarray(inputs["feat1"], dtype=np.float32)
    feat2 = np.asarray(inputs["feat2"], dtype=np.float32)
    in_maps = [
        {
            "feat1": np.ascontiguousarray(feat1[i]),
            "feat2": np.ascontiguousarray(feat2[i]),
            **shared,
        }
        for i in range(N_CORES)
    ]
    res = run_bass_kernel_spmd(
        nc, in_maps, core_ids=list(range(N_CORES)),
        trace=trace, **(trace_kwargs or {}),
    )
    out = np.stack([res.results[i]["out"] for i in range(N_CORES)], axis=0)
    return out, res


def kernel(**inputs) -> np.ndarray:
    out, _ = run(inputs)
    return out
